# revision 45
# baseline (speedup 1.0000x reference)
"""Trainium2 Bass kernel for nn_AdaMLP (MoE routing, 64 experts, 2-layer MLP).

Strategy: expert-parallel over 8 NeuronCores; core i owns experts
[8i, 8i+8). The host groups slots by expert (the MoE dispatch), pads
each group to capacity C, and ships per core:
  - the 8 experts' weights quantized to fp8 e3m4 with per-output-channel
    scales (layer-1 scales folded into layer-2 weights, layer-2 scales
    applied on the PSUM->SBUF output op), clip factor per channel chosen
    to minimize weight MSE,
  - transposed slot groups xT in fp16,
  - per-expert output scale/bias columns in f32.
Each core computes, per expert:  H^T = relu(W1q^T-blocks @ xT),
Out^T = s2 * (W2q-blocks @ H^T) + b2, with the fp8 weights as the
stationary matmul operand.  fp8 weights halve the dominant HBM weight
stream (8.4 MB -> 4.2 MB per core) vs bf16; rel err ~1.8e-2 vs the f32
reference (gate 2e-2), deterministic for a fixed input set.

Schedule (v3, from trace analysis): one merged (w1|w2) 512 KB DMA per
expert on the Sync HWDGE ring, in consumption order, 4 KB elements
throughout — the ring's early throughput scales with element size, and
every alternative tried (pair-merged splits, per-layer splits, a second
weight ring on the Act engine or on GpSimd SWDGE) slowed the stream by
more than its finer gating won back. xt/sb ride the Act ring, whose
~2 us first-data lag hides behind the weight ring's e0 transfer + the
~0.9 us DMA-completion semaphore latency. walrus gets
--min-num-dma-engines-for-dge=16 so the DGE engages all 16 DMA engines
from the first descriptor batch. The expert loop is software-pipelined
via tile_wait_until floors so the in-order PE runs L1(e+1) while
Vector relus expert e. ALL dequant runs on Vector (tensor_scalar mult
by the s2 column): a single nc.scalar.activation would make codegen
insert an ACT_TABLE_LOAD whose ~32 KB fetch occupies the Act DMA path
for ~2.9 us at startup, delaying xt by ~2 us. The final store is split
per oc chunk across the Sync and Act rings so oc0's data moves while
Vector still dequants oc1. The TileContext exit's semaphore
RANGE_CLEAR + second all-engine barrier are patched out (the NEFF exit
sequence resets the whole 256-sem space anyway).

Remaining exec time is dominated by fixed NEFF protocol: ~5.3 us
preamble (engine start barriers + iram/DGE-table loads) and ~7 us exit
sequence (codegen's unconditional 256-semaphore reset storm split
~51/engine, PE slowest at ~115 cyc each) around a ~15 us body that is
within ~2 us of its stream-bound floor.
"""

import numpy as np

P = 128                    # SBUF partitions
DIM = 256                  # slot dim
R = 1024                   # hidden dim
E = 64                     # num experts
NCORES = 8
EPC = E // NCORES          # experts per core
DC = DIM // P              # layer-1 contraction chunks (2)
RC = R // P                # r chunks (8)
OC = DIM // P              # output dim chunks (2)
W1C = DC * R               # w1 columns per expert (2048)
W2C = RC * DIM             # w2 columns per expert (2048)
WCOLS = W1C + W2C          # weight columns per expert (4096)

# fp8 e3m4 weight storage roughly halves the (dominant) weight-table DMA
# traffic vs bf16; measured rel err ~1.8e-2 vs the f32 reference (inside
# the 2e-2 gate). Set False for the bf16 fallback (~3.3e-3).
USE_FP8 = True
SHRINK_SEMS = True

import os
# Ask the DGE for all 16 DMA engines from the first descriptor batch — the
# weight ring otherwise ramps from ~325 GB/s toward ~500 over the first few
# us of the stream.
WALRUS_EXTRA = os.environ.get(
    "KQ_WALRUS_EXTRA", "--min-num-dma-engines-for-dge=16"
)
SKIP_TILE_CLEANUP = os.environ.get("KQ_TILE_CLEANUP", "0") != "1"

_GRAPH_CACHE: dict = {}


def _build_graph(C: int, use_fp8: bool):
    import concourse.bacc as bacc
    import concourse.bass as bass_mod
    import concourse.tile as tile
    from concourse import mybir

    # Shrink the kernel semaphore range (the walrus codegen epilogue's
    # 256-sem reset storm is NOT affected by this — it is unconditional —
    # but a compact kernel range keeps sem allocation stable).
    if SHRINK_SEMS:
        bass_mod.get_kernel_semaphore_range = lambda: range(150, 198)
        import concourse.bass_utils as bu

        if not getattr(bu.get_walrus_args, "_max_sem_patch", False):
            orig_gwa = bu.get_walrus_args

            def _gwa(*a, **kw):
                extra = ["--max-sem-num=198"]
                if WALRUS_EXTRA:
                    extra.extend(WALRUS_EXTRA.split())
                return [*orig_gwa(*a, **kw), *extra]

            _gwa._max_sem_patch = True
            bu.get_walrus_args = _gwa

    f32 = mybir.dt.float32
    wdt = mybir.dt.float8e3 if use_fp8 else mybir.dt.bfloat16
    xdt = mybir.dt.float16 if use_fp8 else mybir.dt.bfloat16

    mx = mybir.AluOpType.max
    mm = mybir.AluOpType.mult

    # Trim the TileContext exit sequence: keep the queue-drain (output DMAs
    # must land before the NEFF exits) and one barrier, but skip the sem
    # RANGE_CLEAR + dma_reset + second barrier — the NEFF's own exit
    # sequence resets the entire 256-sem space right after anyway.
    if SKIP_TILE_CLEANUP and not getattr(
        tile.TileContext._drain_and_barrier, "_skip_cleanup", False
    ):
        from concourse.vector_clock import ScopedClock

        def _drain_and_barrier(self, tick_clock, wait_clock):
            drain_inst = self.nc.sync.drain()
            wait_clock.add_sem_waits(
                drain_inst.ins, ScopedClock({None: tick_clock.global_clock})
            )
            self.nc.all_engine_barrier()
            popped = self.nc._tile_sem_poison_stack.pop()
            assert popped is self._sem_poison
            sems = list(self.sems.allocated().values())
            sem_nums = [s.num for s in sems]
            self.nc._state.prepend_free_semaphores(sem_nums)
            for poison_set in self.nc._tile_sem_poison_stack:
                poison_set.update(sem_nums)

        _drain_and_barrier._skip_cleanup = True
        tile.TileContext._drain_and_barrier = _drain_and_barrier

    nc = bacc.Bacc(None, target_bir_lowering=False)
    xt_ext = nc.declare_dram_parameter("xt", [P, DC * EPC * C], xdt, isOutput=False)
    # flat weight layout: per partition row, [w1(e0)|w2(e0)|w1(e1)|...]
    wg_ext = nc.declare_dram_parameter("wg", [P, EPC * WCOLS], wdt, isOutput=False)
    # per-expert output scale+bias columns: [s2 | b2] per oc chunk
    sb_ext = nc.declare_dram_parameter("sb", [P, EPC * OC * 2], f32, isOutput=False)
    out_ext = nc.declare_dram_parameter("out", [P, EPC * OC * C], f32, isOutput=True)

    with tile.TileContext(nc) as tc:
        with (
            tc.tile_pool(name="xpool", bufs=1) as xpool,
            tc.tile_pool(name="wpool", bufs=1) as wpool,
            tc.tile_pool(name="hpool", bufs=4) as hpool,
            tc.tile_pool(name="opool", bufs=1) as opool,
            tc.tile_pool(name="ps1pool", bufs=3, space="PSUM") as ps1pool,
            tc.tile_pool(name="ps2pool", bufs=3, space="PSUM") as ps2pool,
            tc.tile_pool(name="ps2fpool", bufs=1, space="PSUM") as ps2fpool,
        ):
            xt = xpool.tile([P, DC * EPC * C], xdt)
            sb = xpool.tile([P, EPC * OC * 2], f32)
            # Experts whose weight DMA is split [w1], [w2]. Splitting EARLY
            # experts hurts (2 KB-element halves slow the ring ramp by more
            # than the finer gating wins — measured on e1), but e7 sits at
            # the ring TAIL where all DMA engines are already engaged: its
            # w1 half completes ~0.5 us before the merged DMA would, letting
            # the serial L1(e7)->relu->L2(e7) end-chain start sooner.
            split = set() if os.environ.get("KQ_NOSPLIT7", "0") == "1" else {EPC - 1}
            wtiles = {}
            for e in range(EPC):
                if e in split:
                    w1t = wpool.tile([P, W1C], wdt, name=f"w1t{e}")
                    w2t = wpool.tile([P, W2C], wdt, name=f"w2t{e}")
                    wtiles[e] = (w1t, w2t)
                else:
                    wt = wpool.tile([P, WCOLS], wdt, name=f"wt{e}")
                    wtiles[e] = (wt, None)

            # Weight stream: one merged (w1|w2) 512 KB DMA per expert, all
            # on the Sync HWDGE ring in consumption order (4 KB elements).
            # Exception: e1 is split into [w1],[w2] halves (2 KB elements,
            # tolerable mid-ring) so L1(e1) — the longest early PE stall —
            # gates on w1(e1)'s completion ~0.5 us sooner. xt/sb ride the
            # Act ring (its ~2 us first-data lag is hidden behind the
            # weight ring's e0 transfer + sem latency).
            nc.scalar.dma_start(xt[:], xt_ext[:])
            nc.scalar.dma_start(sb[:], sb_ext[:])
            for e in range(EPC):
                c0 = e * WCOLS
                wa, wb = wtiles[e]
                if wb is not None:
                    nc.sync.dma_start(wa[:], wg_ext[:, c0 : c0 + W1C])
                    nc.sync.dma_start(wb[:], wg_ext[:, c0 + W1C : c0 + WCOLS])
                else:
                    nc.sync.dma_start(wa[:], wg_ext[:, c0 : c0 + WCOLS])

            def w1_slice(e, dc_i, rc_i):
                c = dc_i * R + rc_i * P
                return wtiles[e][0][:, c : c + P]

            def w2_slice(e, rc_i, oc_i):
                wa, wb = wtiles[e]
                c = rc_i * DIM + oc_i * P
                if wb is not None:
                    return wb[:, c : c + P]
                return wa[:, W1C + c : W1C + c + P]

            # single output staging tile; experts 0..6 stored in one bulk
            # DMA during expert 7's compute, expert 7 split per-oc across
            # Sync and Scalar so dequant and store of the halves overlap.
            out_sb = opool.tile([P, EPC * OC * C], f32)

            hs = {}

            def layer2(e):
                # layer 2: Out^T[dim,:] = sum_r W2[r, dim-block] . H^T[r, :]
                # The dequant of each oc chunk is emitted right after that
                # chunk's 8 accumulating matmuls so it gates on just those
                # PSUM writes — for the last expert, oc0's dequant + store
                # overlap oc1's matmuls instead of waiting for all of L2.
                # Dequant runs on Vector (tensor_scalar mult by the s2
                # column; b2 == 0, checked on host). Scalar must emit NO
                # activation ops at all: a single nc.scalar.activation makes
                # codegen insert an ACT_TABLE_LOAD whose ~32 KB table fetch
                # occupies the Act DMA path for ~2.9 us at startup, delaying
                # xt/sb by ~2 us.
                h = hs.pop(e)
                base = e * OC * C
                if e == EPC - 1:
                    # Final expert: each oc chunk gets its OWN PSUM tile and
                    # its dequant + store are emitted right after that
                    # chunk's 8 matmuls. Separate tiles avoid the tile-level
                    # write-after-read that serialized oc1's matmuls behind
                    # deq(oc0) when both chunks shared one tile, and the
                    # interleaved emission gives the dequant a wait on just
                    # its own chunk's writes (wait thresholds follow
                    # emission position). oc0's dequant + Sync-ring store
                    # then overlap oc1's matmuls on the critical tail.
                    for oc_i in range(OC):
                        pst = ps2fpool.tile([P, C], f32, name=f"ps2f{oc_i}")
                        for rc_i in range(RC):
                            nc.tensor.matmul(
                                pst[:, :C],
                                w2_slice(e, rc_i, oc_i),
                                h[:, rc_i * C : rc_i * C + C],
                                start=(rc_i == 0),
                                stop=(rc_i == RC - 1),
                            )
                        nc.vector.tensor_scalar(
                            out_sb[:, base + oc_i * C : base + (oc_i + 1) * C],
                            pst[:, :C],
                            sb[:, (e * OC + oc_i) * 2 : (e * OC + oc_i) * 2 + 1],
                            None,
                            mm,
                        )
                        eng = nc.sync if oc_i == 0 else nc.scalar
                        eng.dma_start(
                            out_ext[:, base + oc_i * C : base + (oc_i + 1) * C],
                            out_sb[:, base + oc_i * C : base + (oc_i + 1) * C],
                        )
                    return
                ps2 = ps2pool.tile([P, OC * C], f32)
                for oc_i in range(OC):
                    for rc_i in range(RC):
                        nc.tensor.matmul(
                            ps2[:, oc_i * C : oc_i * C + C],
                            w2_slice(e, rc_i, oc_i),
                            h[:, rc_i * C : rc_i * C + C],
                            start=(rc_i == 0),
                            stop=(rc_i == RC - 1),
                        )
                for oc_i in range(OC):
                    nc.vector.tensor_scalar(
                        out_sb[:, base + oc_i * C : base + (oc_i + 1) * C],
                        ps2[:, oc_i * C : oc_i * C + C],
                        sb[:, (e * OC + oc_i) * 2 : (e * OC + oc_i) * 2 + 1],
                        None,
                        mm,
                    )
                if e == EPC - 2:
                    # store experts 0..6 while expert 7 computes; only
                    # e7's store rides the critical tail.
                    nc.scalar.dma_start(
                        out_ext[:, : (EPC - 1) * OC * C],
                        out_sb[:, : (EPC - 1) * OC * C],
                    )

            # Software pipeline: the PE queue is in-order, so L1(e) must be
            # SCHEDULED before L2(e-1) — the PE then runs L1(e) while Vector
            # does relu(e-1) instead of stalling. Emission order alone does
            # not guarantee this (the Tile scheduler re-simulates and its
            # DMA model makes the weight arrivals look later than they
            # land), so pace the schedule explicitly: L1(e) floored at the
            # stream cadence, L2(e-1) floored just after L1(e).
            for e in range(EPC):
                with tc.tile_wait_until(0.010 + 0.003 * e):
                    # layer 1: H^T[r,:] = sum_d W1[d, r-block] . xT[d, :]
                    # 8 accumulation groups at offsets of one PSUM tile.
                    ps1 = ps1pool.tile([P, RC * C], f32)
                    for rc_i in range(RC):
                        for dc_i in range(DC):
                            nc.tensor.matmul(
                                ps1[:, rc_i * C : rc_i * C + C],
                                w1_slice(e, dc_i, rc_i),
                                xt[:, (dc_i * EPC + e) * C : (dc_i * EPC + e) * C + C],
                                start=(dc_i == 0),
                                stop=(dc_i == DC - 1),
                            )
                    # single fused relu over all 8 chunks (b1 == 0; checked
                    # on host), on Vector. (Splitting it in halves costs
                    # +270 cyc of fixed op overhead and the scheduler's
                    # emission-position wait thresholds void the overlap.)
                    h = hpool.tile([P, RC * C], xdt)
                    nc.vector.tensor_scalar(h[:], ps1[:], 0.0, None, mx)
                    hs[e] = h
                if e >= 1:
                    with tc.tile_wait_until(0.011 + 0.003 * e):
                        layer2(e - 1)
            with tc.tile_wait_until(0.011 + 0.003 * EPC):
                layer2(EPC - 1)
    nc.compile()
    return nc


def _get_graph(C: int, use_fp8: bool):
    key = (C, use_fp8)
    if key not in _GRAPH_CACHE:
        _GRAPH_CACHE[key] = _build_graph(C, use_fp8)
    return _GRAPH_CACHE[key]


def _quant_e3m4_chan(w, np_e3m4):
    """Quantize w [n_chan along last axis] to e3m4 with per-channel scale;
    clip factor per channel picked from a small grid to minimize MSE.
    w: (..., K, N) quantized per-column-N over axis -2. Returns (q, s)."""
    amax = np.abs(w).max(axis=-2, keepdims=True)
    amax = np.maximum(amax, 1e-30)
    best_err = None
    best_q = None
    best_s = None
    for g in (1.0, 1.05, 1.1, 1.2, 1.35, 1.5):
        s = amax * (g / 15.5)
        q = np.clip(w / s, -15.5, 15.5).astype(np_e3m4)
        err = ((q.astype(np.float32) * s - w) ** 2).sum(axis=-2, keepdims=True)
        if best_err is None:
            best_err, best_q, best_s = err, q, s
        else:
            m = err < best_err
            best_err = np.where(m, err, best_err)
            best_q = np.where(np.broadcast_to(m, q.shape), q, best_q)
            best_s = np.where(m, s, best_s)
    return best_q, best_s[..., 0, :]


def _run(inputs: dict, trace: bool = False, trace_cores=None, use_bf16=None,
         use_fp8=None, **spmd_kwargs):
    from concourse.bass_utils import run_bass_kernel_spmd
    import ml_dtypes

    if use_fp8 is None:
        use_fp8 = USE_FP8 and not use_bf16

    if use_fp8:
        wdt_np = ml_dtypes.float8_e3m4
        xdt_np = np.float16
    else:
        wdt_np = ml_dtypes.bfloat16
        xdt_np = ml_dtypes.bfloat16

    slots = np.asarray(inputs["slots"], np.float32)
    w1 = np.asarray(inputs["w1"], np.float32)
    b1 = np.asarray(inputs["b1"], np.float32)
    w2 = np.asarray(inputs["w2"], np.float32)
    b2 = np.asarray(inputs["b2"], np.float32)
    indices = np.asarray(inputs["indices"]).astype(np.int64)

    B, K, D = slots.shape
    assert D == DIM and w1.shape == (E, DIM, R) and w2.shape == (E, R, DIM)
    assert not b1.any(), "nonzero b1 needs the per-chunk bias path"
    assert not b2.any(), "nonzero b2 needs the tensor_scalar output path"
    X = slots.reshape(B * K, DIM)
    idx = indices.reshape(B * K)

    counts = np.bincount(idx, minlength=E)
    # exact capacity: C appears in the matmul moving width, the relu/dequant
    # widths, and the xt/out DMA bytes — padding it to a multiple of 16
    # wastes ~25% of each for this input distribution (max count 25).
    cround = int(os.environ.get("KQ_CROUND", "16"))
    C = max(int(counts.max()), 8)
    C = ((C + cround - 1) // cround) * cround

    if use_fp8:
        # per-channel-r scales for w1; fold s1 into w2 rows; per-channel-d
        # scales for w2 applied on-device via the output tensor_scalar.
        w1q, s1 = _quant_e3m4_chan(w1, wdt_np)          # (E,D,R), (E,R)
        w2p = w2 * s1[:, :, None]
        w2q, s2 = _quant_e3m4_chan(w2p, wdt_np)          # (E,R,D), (E,D)
    else:
        w1q = w1.astype(wdt_np)
        w2q = w2.astype(wdt_np)
        s2 = np.ones((E, DIM), np.float32)

    in_maps = []
    pos_lists = []
    for core in range(NCORES):
        xt = np.zeros((P, DC * EPC * C), xdt_np)
        wg = np.empty((P, EPC * WCOLS), wdt_np)
        sb = np.zeros((P, EPC * OC * 2), np.float32)
        core_pos = []
        for e in range(EPC):
            g = core * EPC + e
            pos = np.nonzero(idx == g)[0]
            core_pos.append(pos)
            n = len(pos)
            if n:
                xeT = X[pos].T.astype(xdt_np)  # [DIM, n]
                for dc_i in range(DC):
                    xt[:, (dc_i * EPC + e) * C : (dc_i * EPC + e) * C + n] = (
                        xeT[dc_i * P : (dc_i + 1) * P]
                    )
            wg[:, e * WCOLS : e * WCOLS + W1C] = (
                w1q[g].reshape(DC, P, R).transpose(1, 0, 2).reshape(P, W1C)
            )
            wg[:, e * WCOLS + W1C : (e + 1) * WCOLS] = (
                w2q[g].reshape(RC, P, DIM).transpose(1, 0, 2).reshape(P, W2C)
            )
            for oc_i in range(OC):
                k = (e * OC + oc_i) * 2
                sb[:, k] = s2[g, oc_i * P : (oc_i + 1) * P]
                sb[:, k + 1] = b2[g, oc_i * P : (oc_i + 1) * P]
        in_maps.append({"xt": xt, "wg": wg, "sb": sb})
        pos_lists.append(core_pos)

    nc = _get_graph(C, use_fp8)
    res = run_bass_kernel_spmd(
        nc, in_maps, core_ids=list(range(NCORES)), trace=trace,
        trace_cores=trace_cores, **spmd_kwargs,
    )

    out_flat = np.zeros((B * K, DIM), np.float32)
    for core in range(NCORES):
        o = res.results[core]["out"]  # [P, EPC*OC*C]
        for e in range(EPC):
            pos = pos_lists[core][e]
            n = len(pos)
            if n == 0:
                continue
            blk = np.empty((n, DIM), np.float32)
            for oc_i in range(OC):
                cols = o[:, (e * OC + oc_i) * C : (e * OC + oc_i) * C + n]
                blk[:, oc_i * P : (oc_i + 1) * P] = cols.T
            out_flat[pos] = blk
    return out_flat.reshape(B, K, DIM), res


def kernel(**inputs) -> np.ndarray:
    out, _ = _run(inputs)
    return out


# revision 47
# speedup vs baseline: 1.0811x; 1.0811x over previous
"""Trainium2 Bass kernel for nn_AdaMLP (MoE routing, 64 experts, 2-layer MLP).

Strategy: expert-parallel over 8 NeuronCores; core i owns experts
[8i, 8i+8). The host groups slots by expert (the MoE dispatch), pads
each group to capacity C, and ships per core:
  - the 8 experts' weights quantized to fp8 e3m4 with per-output-channel
    scales (layer-1 scales folded into layer-2 weights, layer-2 scales
    applied on the PSUM->SBUF output op), clip factor per channel chosen
    to minimize weight MSE,
  - transposed slot groups xT in fp16,
  - per-expert output scale/bias columns in f32.
Each core computes, per expert:  H^T = relu(W1q^T-blocks @ xT),
Out^T = s2 * (W2q-blocks @ H^T) + b2, with the fp8 weights as the
stationary matmul operand.  fp8 weights halve the dominant HBM weight
stream (8.4 MB -> 4.2 MB per core) vs bf16; rel err ~1.8e-2 vs the f32
reference (gate 2e-2), deterministic for a fixed input set.

Schedule (v3, from trace analysis): one merged (w1|w2) 512 KB DMA per
expert on the Sync HWDGE ring, in consumption order, 4 KB elements
throughout — the ring's early throughput scales with element size, and
every alternative tried (pair-merged splits, per-layer splits, a second
weight ring on the Act engine or on GpSimd SWDGE) slowed the stream by
more than its finer gating won back. xt/sb ride the Act ring, whose
~2 us first-data lag hides behind the weight ring's e0 transfer + the
~0.9 us DMA-completion semaphore latency. walrus gets
--min-num-dma-engines-for-dge=16 so the DGE engages all 16 DMA engines
from the first descriptor batch. The expert loop is software-pipelined
via tile_wait_until floors so the in-order PE runs L1(e+1) while
Vector relus expert e. ALL dequant runs on Vector (tensor_scalar mult
by the s2 column): a single nc.scalar.activation would make codegen
insert an ACT_TABLE_LOAD whose ~32 KB fetch occupies the Act DMA path
for ~2.9 us at startup, delaying xt by ~2 us. The final store is split
per oc chunk across the Sync and Act rings so oc0's data moves while
Vector still dequants oc1. The TileContext exit's semaphore
RANGE_CLEAR + second all-engine barrier are patched out (the NEFF exit
sequence resets the whole 256-sem space anyway).

Remaining exec time is dominated by fixed NEFF protocol: ~5.3 us
preamble (engine start barriers + iram/DGE-table loads) and ~7 us exit
sequence (codegen's unconditional 256-semaphore reset storm split
~51/engine, PE slowest at ~115 cyc each) around a ~15 us body that is
within ~2 us of its stream-bound floor.
"""

import numpy as np

P = 128                    # SBUF partitions
DIM = 256                  # slot dim
R = 1024                   # hidden dim
E = 64                     # num experts
NCORES = 8
EPC = E // NCORES          # experts per core
DC = DIM // P              # layer-1 contraction chunks (2)
RC = R // P                # r chunks (8)
OC = DIM // P              # output dim chunks (2)
W1C = DC * R               # w1 columns per expert (2048)
W2C = RC * DIM             # w2 columns per expert (2048)
WCOLS = W1C + W2C          # weight columns per expert (4096)

# fp8 e3m4 weight storage roughly halves the (dominant) weight-table DMA
# traffic vs bf16; measured rel err ~1.8e-2 vs the f32 reference (inside
# the 2e-2 gate). Set False for the bf16 fallback (~3.3e-3).
USE_FP8 = True
SHRINK_SEMS = True

import os
# Ask the DGE for all 16 DMA engines from the first descriptor batch — the
# weight ring otherwise ramps from ~325 GB/s toward ~500 over the first few
# us of the stream.
WALRUS_EXTRA = os.environ.get(
    "KQ_WALRUS_EXTRA", "--min-num-dma-engines-for-dge=16"
)
SKIP_TILE_CLEANUP = os.environ.get("KQ_TILE_CLEANUP", "0") != "1"

_GRAPH_CACHE: dict = {}


def _build_graph(C: int, use_fp8: bool):
    import concourse.bacc as bacc
    import concourse.bass as bass_mod
    import concourse.tile as tile
    from concourse import mybir

    # Shrink the kernel semaphore range (the walrus codegen epilogue's
    # 256-sem reset storm is NOT affected by this — it is unconditional —
    # but a compact kernel range keeps sem allocation stable).
    if SHRINK_SEMS:
        bass_mod.get_kernel_semaphore_range = lambda: range(150, 198)
        import concourse.bass_utils as bu

        if not getattr(bu.get_walrus_args, "_max_sem_patch", False):
            orig_gwa = bu.get_walrus_args

            def _gwa(*a, **kw):
                extra = ["--max-sem-num=198"]
                if WALRUS_EXTRA:
                    extra.extend(WALRUS_EXTRA.split())
                return [*orig_gwa(*a, **kw), *extra]

            _gwa._max_sem_patch = True
            bu.get_walrus_args = _gwa

    f32 = mybir.dt.float32
    wdt = mybir.dt.float8e3 if use_fp8 else mybir.dt.bfloat16
    xdt = mybir.dt.float16 if use_fp8 else mybir.dt.bfloat16

    mx = mybir.AluOpType.max
    mm = mybir.AluOpType.mult

    # Trim the TileContext exit sequence: keep the queue-drain (output DMAs
    # must land before the NEFF exits) and one barrier, but skip the sem
    # RANGE_CLEAR + dma_reset + second barrier — the NEFF's own exit
    # sequence resets the entire 256-sem space right after anyway.
    if SKIP_TILE_CLEANUP and not getattr(
        tile.TileContext._drain_and_barrier, "_skip_cleanup", False
    ):
        from concourse.vector_clock import ScopedClock

        def _drain_and_barrier(self, tick_clock, wait_clock):
            drain_inst = self.nc.sync.drain()
            wait_clock.add_sem_waits(
                drain_inst.ins, ScopedClock({None: tick_clock.global_clock})
            )
            self.nc.all_engine_barrier()
            popped = self.nc._tile_sem_poison_stack.pop()
            assert popped is self._sem_poison
            sems = list(self.sems.allocated().values())
            sem_nums = [s.num for s in sems]
            self.nc._state.prepend_free_semaphores(sem_nums)
            for poison_set in self.nc._tile_sem_poison_stack:
                poison_set.update(sem_nums)

        _drain_and_barrier._skip_cleanup = True
        tile.TileContext._drain_and_barrier = _drain_and_barrier

    nc = bacc.Bacc(None, target_bir_lowering=False)
    xt_ext = nc.declare_dram_parameter("xt", [P, DC * EPC * C], xdt, isOutput=False)
    # flat weight layout: per partition row, [w1(e0)|w2(e0)|w1(e1)|...]
    wg_ext = nc.declare_dram_parameter("wg", [P, EPC * WCOLS], wdt, isOutput=False)
    # per-expert output scale+bias columns: [s2 | b2] per oc chunk
    sb_ext = nc.declare_dram_parameter("sb", [P, EPC * OC * 2], f32, isOutput=False)
    out_ext = nc.declare_dram_parameter("out", [P, EPC * OC * C], f32, isOutput=True)

    with tile.TileContext(nc) as tc:
        with (
            tc.tile_pool(name="xpool", bufs=1) as xpool,
            tc.tile_pool(name="wpool", bufs=1) as wpool,
            tc.tile_pool(name="hpool", bufs=4) as hpool,
            tc.tile_pool(name="opool", bufs=1) as opool,
            tc.tile_pool(name="ps1pool", bufs=3, space="PSUM") as ps1pool,
            tc.tile_pool(name="ps2pool", bufs=3, space="PSUM") as ps2pool,
            tc.tile_pool(name="ps2fpool", bufs=1, space="PSUM") as ps2fpool,
        ):
            xt = xpool.tile([P, DC * EPC * C], xdt)
            sb = xpool.tile([P, EPC * OC * 2], f32)
            # Experts whose weight DMA is split [w1], [w2]. Splitting EARLY
            # experts hurts (2 KB-element halves slow the ring ramp by more
            # than the finer gating wins — measured on e1), but e7 sits at
            # the ring TAIL where all DMA engines are already engaged: its
            # w1 half completes ~0.5 us before the merged DMA would, letting
            # the serial L1(e7)->relu->L2(e7) end-chain start sooner.
            split = set() if os.environ.get("KQ_NOSPLIT7", "0") == "1" else {EPC - 1}
            wtiles = {}
            for e in range(EPC):
                if e in split:
                    w1t = wpool.tile([P, W1C], wdt, name=f"w1t{e}")
                    w2t = wpool.tile([P, W2C], wdt, name=f"w2t{e}")
                    wtiles[e] = (w1t, w2t)
                else:
                    wt = wpool.tile([P, WCOLS], wdt, name=f"wt{e}")
                    wtiles[e] = (wt, None)

            # Weight stream: one merged (w1|w2) 512 KB DMA per expert, all
            # on the Sync HWDGE ring in consumption order (4 KB elements).
            # Exception: e1 is split into [w1],[w2] halves (2 KB elements,
            # tolerable mid-ring) so L1(e1) — the longest early PE stall —
            # gates on w1(e1)'s completion ~0.5 us sooner. xt/sb ride the
            # Act ring (its ~2 us first-data lag is hidden behind the
            # weight ring's e0 transfer + sem latency).
            nc.scalar.dma_start(xt[:], xt_ext[:])
            nc.scalar.dma_start(sb[:], sb_ext[:])
            for e in range(EPC):
                c0 = e * WCOLS
                wa, wb = wtiles[e]
                if wb is not None:
                    nc.sync.dma_start(wa[:], wg_ext[:, c0 : c0 + W1C])
                    nc.sync.dma_start(wb[:], wg_ext[:, c0 + W1C : c0 + WCOLS])
                else:
                    nc.sync.dma_start(wa[:], wg_ext[:, c0 : c0 + WCOLS])

            def w1_slice(e, dc_i, rc_i):
                c = dc_i * R + rc_i * P
                return wtiles[e][0][:, c : c + P]

            def w2_slice(e, rc_i, oc_i):
                wa, wb = wtiles[e]
                c = rc_i * DIM + oc_i * P
                if wb is not None:
                    return wb[:, c : c + P]
                return wa[:, W1C + c : W1C + c + P]

            # single output staging tile; experts 0..6 stored in one bulk
            # DMA during expert 7's compute, expert 7 split per-oc across
            # Sync and Scalar so dequant and store of the halves overlap.
            out_sb = opool.tile([P, EPC * OC * C], f32)

            hs = {}

            def layer2(e):
                # layer 2: Out^T[dim,:] = sum_r W2[r, dim-block] . H^T[r, :]
                # The dequant of each oc chunk is emitted right after that
                # chunk's 8 accumulating matmuls so it gates on just those
                # PSUM writes — for the last expert, oc0's dequant + store
                # overlap oc1's matmuls instead of waiting for all of L2.
                # Dequant runs on Vector (tensor_scalar mult by the s2
                # column; b2 == 0, checked on host). Scalar must emit NO
                # activation ops at all: a single nc.scalar.activation makes
                # codegen insert an ACT_TABLE_LOAD whose ~32 KB table fetch
                # occupies the Act DMA path for ~2.9 us at startup, delaying
                # xt/sb by ~2 us.
                h = hs.pop(e)
                base = e * OC * C
                if e == EPC - 1:
                    # Final expert: each oc chunk gets its OWN PSUM tile and
                    # its dequant + store are emitted right after that
                    # chunk's 8 matmuls. Separate tiles avoid the tile-level
                    # write-after-read that serialized oc1's matmuls behind
                    # deq(oc0) when both chunks shared one tile, and the
                    # interleaved emission gives the dequant a wait on just
                    # its own chunk's writes (wait thresholds follow
                    # emission position). oc0's dequant + Sync-ring store
                    # then overlap oc1's matmuls on the critical tail.
                    for oc_i in range(OC):
                        pst = ps2fpool.tile([P, C], f32, name=f"ps2f{oc_i}")
                        for rc_i in range(RC):
                            nc.tensor.matmul(
                                pst[:, :C],
                                w2_slice(e, rc_i, oc_i),
                                h[:, rc_i * C : rc_i * C + C],
                                start=(rc_i == 0),
                                stop=(rc_i == RC - 1),
                            )
                        nc.vector.tensor_scalar(
                            out_sb[:, base + oc_i * C : base + (oc_i + 1) * C],
                            pst[:, :C],
                            sb[:, (e * OC + oc_i) * 2 : (e * OC + oc_i) * 2 + 1],
                            None,
                            mm,
                        )
                        eng = nc.sync if oc_i == 0 else nc.scalar
                        eng.dma_start(
                            out_ext[:, base + oc_i * C : base + (oc_i + 1) * C],
                            out_sb[:, base + oc_i * C : base + (oc_i + 1) * C],
                        )
                    return
                ps2 = ps2pool.tile([P, OC * C], f32)
                for oc_i in range(OC):
                    for rc_i in range(RC):
                        nc.tensor.matmul(
                            ps2[:, oc_i * C : oc_i * C + C],
                            w2_slice(e, rc_i, oc_i),
                            h[:, rc_i * C : rc_i * C + C],
                            start=(rc_i == 0),
                            stop=(rc_i == RC - 1),
                        )
                for oc_i in range(OC):
                    nc.vector.tensor_scalar(
                        out_sb[:, base + oc_i * C : base + (oc_i + 1) * C],
                        ps2[:, oc_i * C : oc_i * C + C],
                        sb[:, (e * OC + oc_i) * 2 : (e * OC + oc_i) * 2 + 1],
                        None,
                        mm,
                    )
                if e == EPC - 3:
                    # store experts 0..5 as soon as deq(e5) lands, on Sync
                    # (idle once the weight stream is issued) — off the
                    # critical tail entirely.
                    nc.sync.dma_start(
                        out_ext[:, : (EPC - 2) * OC * C],
                        out_sb[:, : (EPC - 2) * OC * C],
                    )
                if e == EPC - 2:
                    # e6's small store follows on Sync; keeping it separate
                    # from the e0..e5 bulk shrinks the descriptor-gen that
                    # sits in front of e7-oc0's store on the Sync queue.
                    # (On Scalar either store would delay e7-oc1's issue.)
                    nc.sync.dma_start(
                        out_ext[:, (EPC - 2) * OC * C : (EPC - 1) * OC * C],
                        out_sb[:, (EPC - 2) * OC * C : (EPC - 1) * OC * C],
                    )

            # Software pipeline: the PE queue is in-order, so L1(e) must be
            # SCHEDULED before L2(e-1) — the PE then runs L1(e) while Vector
            # does relu(e-1) instead of stalling. Emission order alone does
            # not guarantee this (the Tile scheduler re-simulates and its
            # DMA model makes the weight arrivals look later than they
            # land), so pace the schedule explicitly: L1(e) floored at the
            # stream cadence, L2(e-1) floored just after L1(e).
            for e in range(EPC):
                with tc.tile_wait_until(0.010 + 0.003 * e):
                    # layer 1: H^T[r,:] = sum_d W1[d, r-block] . xT[d, :]
                    # 8 accumulation groups at offsets of one PSUM tile.
                    ps1 = ps1pool.tile([P, RC * C], f32)
                    for rc_i in range(RC):
                        for dc_i in range(DC):
                            nc.tensor.matmul(
                                ps1[:, rc_i * C : rc_i * C + C],
                                w1_slice(e, dc_i, rc_i),
                                xt[:, (dc_i * EPC + e) * C : (dc_i * EPC + e) * C + C],
                                start=(dc_i == 0),
                                stop=(dc_i == DC - 1),
                            )
                    # single fused relu over all 8 chunks (b1 == 0; checked
                    # on host), on Vector. (Splitting it in halves costs
                    # +270 cyc of fixed op overhead and the scheduler's
                    # emission-position wait thresholds void the overlap.)
                    h = hpool.tile([P, RC * C], xdt)
                    nc.vector.tensor_scalar(h[:], ps1[:], 0.0, None, mx)
                    hs[e] = h
                if e >= 1:
                    with tc.tile_wait_until(0.011 + 0.003 * e):
                        layer2(e - 1)
            with tc.tile_wait_until(0.011 + 0.003 * EPC):
                layer2(EPC - 1)
    nc.compile()
    return nc


def _get_graph(C: int, use_fp8: bool):
    key = (C, use_fp8)
    if key not in _GRAPH_CACHE:
        _GRAPH_CACHE[key] = _build_graph(C, use_fp8)
    return _GRAPH_CACHE[key]


def _quant_e3m4_chan(w, np_e3m4):
    """Quantize w [n_chan along last axis] to e3m4 with per-channel scale;
    clip factor per channel picked from a small grid to minimize MSE.
    w: (..., K, N) quantized per-column-N over axis -2. Returns (q, s)."""
    amax = np.abs(w).max(axis=-2, keepdims=True)
    amax = np.maximum(amax, 1e-30)
    best_err = None
    best_q = None
    best_s = None
    for g in (1.0, 1.05, 1.1, 1.2, 1.35, 1.5):
        s = amax * (g / 15.5)
        q = np.clip(w / s, -15.5, 15.5).astype(np_e3m4)
        err = ((q.astype(np.float32) * s - w) ** 2).sum(axis=-2, keepdims=True)
        if best_err is None:
            best_err, best_q, best_s = err, q, s
        else:
            m = err < best_err
            best_err = np.where(m, err, best_err)
            best_q = np.where(np.broadcast_to(m, q.shape), q, best_q)
            best_s = np.where(m, s, best_s)
    return best_q, best_s[..., 0, :]


def _run(inputs: dict, trace: bool = False, trace_cores=None, use_bf16=None,
         use_fp8=None, **spmd_kwargs):
    from concourse.bass_utils import run_bass_kernel_spmd
    import ml_dtypes

    if use_fp8 is None:
        use_fp8 = USE_FP8 and not use_bf16

    if use_fp8:
        wdt_np = ml_dtypes.float8_e3m4
        xdt_np = np.float16
    else:
        wdt_np = ml_dtypes.bfloat16
        xdt_np = ml_dtypes.bfloat16

    slots = np.asarray(inputs["slots"], np.float32)
    w1 = np.asarray(inputs["w1"], np.float32)
    b1 = np.asarray(inputs["b1"], np.float32)
    w2 = np.asarray(inputs["w2"], np.float32)
    b2 = np.asarray(inputs["b2"], np.float32)
    indices = np.asarray(inputs["indices"]).astype(np.int64)

    B, K, D = slots.shape
    assert D == DIM and w1.shape == (E, DIM, R) and w2.shape == (E, R, DIM)
    assert not b1.any(), "nonzero b1 needs the per-chunk bias path"
    assert not b2.any(), "nonzero b2 needs the tensor_scalar output path"
    X = slots.reshape(B * K, DIM)
    idx = indices.reshape(B * K)

    counts = np.bincount(idx, minlength=E)
    # exact capacity: C appears in the matmul moving width, the relu/dequant
    # widths, and the xt/out DMA bytes — padding it to a multiple of 16
    # wastes ~25% of each for this input distribution (max count 25).
    cround = int(os.environ.get("KQ_CROUND", "16"))
    C = max(int(counts.max()), 8)
    C = ((C + cround - 1) // cround) * cround

    if use_fp8:
        # per-channel-r scales for w1; fold s1 into w2 rows; per-channel-d
        # scales for w2 applied on-device via the output tensor_scalar.
        w1q, s1 = _quant_e3m4_chan(w1, wdt_np)          # (E,D,R), (E,R)
        w2p = w2 * s1[:, :, None]
        w2q, s2 = _quant_e3m4_chan(w2p, wdt_np)          # (E,R,D), (E,D)
    else:
        w1q = w1.astype(wdt_np)
        w2q = w2.astype(wdt_np)
        s2 = np.ones((E, DIM), np.float32)

    in_maps = []
    pos_lists = []
    for core in range(NCORES):
        xt = np.zeros((P, DC * EPC * C), xdt_np)
        wg = np.empty((P, EPC * WCOLS), wdt_np)
        sb = np.zeros((P, EPC * OC * 2), np.float32)
        core_pos = []
        for e in range(EPC):
            g = core * EPC + e
            pos = np.nonzero(idx == g)[0]
            core_pos.append(pos)
            n = len(pos)
            if n:
                xeT = X[pos].T.astype(xdt_np)  # [DIM, n]
                for dc_i in range(DC):
                    xt[:, (dc_i * EPC + e) * C : (dc_i * EPC + e) * C + n] = (
                        xeT[dc_i * P : (dc_i + 1) * P]
                    )
            wg[:, e * WCOLS : e * WCOLS + W1C] = (
                w1q[g].reshape(DC, P, R).transpose(1, 0, 2).reshape(P, W1C)
            )
            wg[:, e * WCOLS + W1C : (e + 1) * WCOLS] = (
                w2q[g].reshape(RC, P, DIM).transpose(1, 0, 2).reshape(P, W2C)
            )
            for oc_i in range(OC):
                k = (e * OC + oc_i) * 2
                sb[:, k] = s2[g, oc_i * P : (oc_i + 1) * P]
                sb[:, k + 1] = b2[g, oc_i * P : (oc_i + 1) * P]
        in_maps.append({"xt": xt, "wg": wg, "sb": sb})
        pos_lists.append(core_pos)

    nc = _get_graph(C, use_fp8)
    res = run_bass_kernel_spmd(
        nc, in_maps, core_ids=list(range(NCORES)), trace=trace,
        trace_cores=trace_cores, **spmd_kwargs,
    )

    out_flat = np.zeros((B * K, DIM), np.float32)
    for core in range(NCORES):
        o = res.results[core]["out"]  # [P, EPC*OC*C]
        for e in range(EPC):
            pos = pos_lists[core][e]
            n = len(pos)
            if n == 0:
                continue
            blk = np.empty((n, DIM), np.float32)
            for oc_i in range(OC):
                cols = o[:, (e * OC + oc_i) * C : (e * OC + oc_i) * C + n]
                blk[:, oc_i * P : (oc_i + 1) * P] = cols.T
            out_flat[pos] = blk
    return out_flat.reshape(B, K, DIM), res


def kernel(**inputs) -> np.ndarray:
    out, _ = _run(inputs)
    return out
